# revision 14
# baseline (speedup 1.0000x reference)
"""ArcFace loss on 8 TRN2 NeuronCores — class-parallel (C=64000 over 8 cores).

No device collectives: each core emits tiny partials (its local exp-sums
over classes and masked target logit per batch row); the host
gathers/unshards the 8 partials and finishes the O(B) scalar loss math
(arccos/cos/log over 512 values) — the same data an AllReduce would
have exchanged.

Per core (C_local=8000 padded to 8192 = 8 superchunks * 1024):
  - host pre-normalizes W rows and x rows, transposes, scales by 16 and
    quantizes to fp8(e4m3, max 240); theta tiles come out of fp8
    DoubleRow matmuls (256-deep contraction each) in PSUM f32
  - superchunks 0-1 run class-major [c,b]: exp(64*theta - 8) to fp8 on
    ACT, class-sum via fp8 DoubleRow ones-matmuls accumulated in PSUM
    [1,512] (PE does these sums)
  - superchunks 2-7 run batch-major [b,c]: exp to bf16 on ACT over
    2-PSUM-bank [128,1024] tiles, class-sum on DVE tensor_reduce
  - x norms (for the exact target logit only): squares on GpSimd,
    reduce on DVE, rsqrt via ACT ln/exp
  - target logits: indirect-DMA gather of normalized W rows (f32),
    multiply on GpSimd + reduce on DVE, masked by ownership
  - pad classes (192, all in the last chunk) contribute exp(0)=1 each
Host: fs = sum_i(fs_bc_i + e^8 * fs_cb_i) - 8*192; t = sum_i tgt_i;
      num = S*cos(arccos(t)+M);
      loss = -mean(num - log(exp(num) + fs - exp(S*t))) in float64.
"""

import json
import math

import numpy as np

S = 64.0
MARG = 0.5
EPS = 1e-7
B, D, C = 512, 512, 64000
NCORES = 8
CL = C // NCORES            # 8000
NSC = 8                     # superchunks of 1024 classes
SCW = 1024
CPAD = NSC * SCW            # 8192
NPAD = CPAD - CL            # 192 zero-pad classes per core
QS = 16.0                   # fp8 pre-scale for both xhat and What
APE = 2                     # superchunks summed on PE ([c,b] layout)
SHIFT = 8.0                 # exp shift for fp8 ex range
S256 = S / (QS * QS)

_MAX_WAITS = 1


def _split_waits(bir_bytes, max_waits=_MAX_WAITS):
    """walrus in this env rejects >1 sync-wait per instruction; spill extras
    onto preceding wait-only EventSemaphore instructions (same engine)."""
    m = json.loads(bir_bytes)
    uid = [0]
    for f in m.get("functions", []):
        for blk in f.get("blocks", []):
            insts = blk.get("instructions", [])
            out = []
            for i in insts:
                si = i.get("sync_info") or {}
                ws = si.get("on_wait") or []
                if len(ws) > max_waits:
                    keep = ws[-max_waits:]
                    extra = ws[:-max_waits]
                    for cs in range(0, len(extra), max_waits):
                        uid[0] += 1
                        out.append({
                            "name": f"WSPLIT-{uid[0]}",
                            "opcode": "EventSemaphore",
                            "engine": i["engine"],
                            "ins": [],
                            "outs": [],
                            "sync_info": {"on_update": [],
                                          "on_wait": extra[cs:cs + max_waits]},
                        })
                    si["on_wait"] = keep
                out.append(i)
            blk["instructions"] = out
    return json.dumps(m).encode()


def _install_birfix():
    from concourse import bass
    if getattr(bass.Bass, "_birfix_installed", False):
        return
    orig = bass.Bass.to_json_bytes

    def to_json_bytes(self, *a, **k):
        return _split_waits(orig(self, *a, **k))

    bass.Bass.to_json_bytes = to_json_bytes
    bass.Bass._birfix_installed = True


def build():
    _install_birfix()
    from concourse import bass, tile, mybir
    from concourse.tile import add_dep_helper

    f32 = mybir.dt.float32
    bf16 = mybir.dt.bfloat16
    fp8 = mybir.dt.float8e4
    i32 = mybir.dt.int32
    AX = mybir.AxisListType
    OP = mybir.AluOpType
    AF = mybir.ActivationFunctionType
    DR = mybir.MatmulPerfMode.DoubleRow

    nc = bass.Bass("TRN2", target_bir_lowering=False, debug=False,
                   num_devices=NCORES)
    wt = nc.declare_dram_parameter("wt", [128, NSC * 4096], fp8,
                                   isOutput=False)
    xt = nc.declare_dram_parameter("xt", [128, 4 * B], fp8, isOutput=False)
    xx = nc.declare_dram_parameter("x", [B, D], f32, isOutput=False)
    wn = nc.declare_dram_parameter("wn", [CL, D], f32, isOutput=False)
    yi = nc.declare_dram_parameter("yi", [128, 4], i32, isOutput=False)
    yv = nc.declare_dram_parameter("yv", [128, 4], f32, isOutput=False)
    out = nc.declare_dram_parameter("out", [128, 8], f32, isOutput=True)
    out2 = nc.declare_dram_parameter("out2", [1, B], f32, isOutput=True)

    last = {}

    def chain(key, inst):
        if key in last:
            add_dep_helper(inst.ins, last[key].ins, False, f"{key} order")
        last[key] = inst
        return inst

    with tile.TileContext(nc) as tc:
        with tc.tile_pool(name="const", bufs=1) as cpool, \
             tc.tile_pool(name="big", bufs=1) as big, \
             tc.tile_pool(name="sm", bufs=1) as sm, \
             tc.tile_pool(name="ex2p", bufs=4) as ex2p, \
             tc.tile_pool(name="ex8p", bufs=8) as ex8p, \
             tc.tile_pool(name="mp2p", bufs=3, space="PSUM") as mp2p, \
             tc.tile_pool(name="sump", bufs=1, space="PSUM") as sump:

            ones_b = cpool.tile([128, 1], bf16, name="ones_b")
            nc.gpsimd.memset(ones_b[:], 1.0)

            # ---- input tiles: xt + W stream first, gather inputs later ----
            xtile = big.tile([128, 4 * B], fp8, name="xtile")
            xr = [sm.tile([128, D], f32, name=f"xr{t}") for t in range(4)]
            idx = sm.tile([128, 4], i32, name="idx")
            yvs = sm.tile([128, 4], f32, name="yvs")
            wtile = [big.tile([128, 4096], fp8, name=f"wt{d}")
                     for d in range(NSC)]

            chain("syn", nc.sync.dma_start(out=xtile[:], in_=xt[:]))
            for d in range(NSC):
                chain("syn", nc.sync.dma_start(
                    out=wtile[d][:], in_=wt[:, 4096 * d:4096 * (d + 1)]))
            for t in range(4):
                chain("syn", nc.sync.dma_start(
                    out=xr[t][:], in_=xx[128 * t:128 * (t + 1), :]))
            chain("syn", nc.sync.dma_start(out=idx[:], in_=yi[:]))
            chain("syn", nc.sync.dma_start(out=yvs[:], in_=yv[:]))

            # views: xv [128k, kt, b];  wv [128k, j, kt, c]
            xv = xtile.rearrange("p (k b) -> p k b", k=4)
            wv = [wtile[d].rearrange("p (s k c) -> p s k c", s=2, k=4)
                  for d in range(NSC)]

            # ---- x row norms (target path only): GpSimd mult, DVE reduce --
            xnsq = sm.tile([128, 4], f32, name="xnsq")
            dum = [sm.tile([128, D], f32, name=f"dum{i}") for i in range(2)]

            def emit_xnorm():
                for t in range(4):
                    chain("gps", nc.gpsimd.tensor_tensor(
                        dum[t % 2][:], xr[t][:], xr[t][:], OP.mult))
                    chain("dve", nc.vector.tensor_reduce(
                        out=xnsq[:, t:t + 1], in_=dum[t % 2][:], axis=AX.X,
                        op=OP.add))

            def emit_xinv():
                xnm = sm.tile([128, 4], f32, name="xnm")
                chain("dve", nc.vector.tensor_scalar_max(xnm[:], xnsq[:],
                                                         1e-30))
                lnx = sm.tile([128, 4], f32, name="lnx")
                chain("act", nc.scalar.activation(out=lnx[:], in_=xnm[:],
                                                  func=AF.Ln))
                xinv = sm.tile([128, 4], f32, name="xinv")
                chain("act", nc.scalar.activation(out=xinv[:], in_=lnx[:],
                                                  func=AF.Exp, scale=-0.5))
                return xinv

            # ---- gather path: normalized W rows (f32) ----
            wsel = sm.tile([128, 4, D], f32, name="wsel")

            def emit_gather_dma():
                for t in range(4):
                    chain("gps", nc.gpsimd.indirect_dma_start(
                        out=wsel[:, t, :], out_offset=None, in_=wn[:],
                        in_offset=bass.IndirectOffsetOnAxis(
                            ap=idx[:, t:t + 1], axis=0)))

            dots = sm.tile([128, 4], f32, name="dots")
            gdum = [sm.tile([128, D], f32, name=f"gdum{i}") for i in range(2)]

            def emit_gather_dots():
                for t in range(4):
                    chain("gps", nc.gpsimd.tensor_tensor(
                        gdum[t % 2][:], xr[t][:], wsel[:, t, :], OP.mult))
                    chain("dve", nc.vector.tensor_reduce(
                        out=dots[:, t:t + 1], in_=gdum[t % 2][:], axis=AX.X,
                        op=OP.add))

            # ---- main loop ----
            sumP = sump.tile([1, B], f32, name="sumP")
            fsacc = [sm.tile([128, NSC - APE], f32, name=f"fsacc{b}")
                     for b in range(4)]

            exq = []    # deferred ones-matmul inputs ([c,b] exp tiles)
            ones_total = APE * 8

            def flush_ones(k=1):
                while exq and k > 0:
                    eo = exq.pop(0)
                    flush_ones.emitted += 1
                    chain("pe", nc.tensor.matmul(
                        sumP[:], lhsT=ones_b[:], rhs=eo,
                        start=(flush_ones.emitted == 1),
                        stop=(flush_ones.emitted == ones_total)))
                    k -= 1
            flush_ones.emitted = 0

            for d in range(NSC):
                if d < APE:
                    # class-major: exp->bf16, class-sum on PE (ones-mm,
                    # deferred so PE never stalls on ACT)
                    for pr in range(4):
                        mp2 = mp2p.tile([128, 2 * B], f32, tag="mp")
                        for kp in range(2):
                            for h in range(2):
                                ci = 2 * pr + h
                                chain("pe", nc.tensor.matmul(
                                    mp2[:, 512 * h:512 * (h + 1)],
                                    lhsT=wv[d][:, ci // 4, 2 * kp:2 * kp + 2,
                                               128 * (ci % 4):
                                               128 * (ci % 4 + 1)],
                                    rhs=xv[:, 2 * kp:2 * kp + 2, :],
                                    start=(kp == 0), stop=(kp == 1),
                                    perf_mode=DR))
                        ex8 = ex8p.tile([128, 2, 512], bf16, tag="ex8")
                        for h in range(2):
                            chain("act", nc.scalar.activation(
                                out=ex8[:, h, :],
                                in_=mp2[:, 512 * h:512 * (h + 1)],
                                func=AF.Exp, scale=S256))
                            exq.append(ex8[:, h, :])
                else:
                    # batch-major: exp->bf16 over 2 banks, sum on DVE
                    for b in range(4):
                        flush_ones(1)
                        mp2 = mp2p.tile([128, 2 * B], f32, tag="mp")
                        for kp in range(2):
                            for j in range(2):
                                chain("pe", nc.tensor.matmul(
                                    mp2[:, 512 * j:512 * (j + 1)],
                                    lhsT=xv[:, 2 * kp:2 * kp + 2,
                                            128 * b:128 * (b + 1)],
                                    rhs=wv[d][:, j, 2 * kp:2 * kp + 2, :],
                                    start=(kp == 0), stop=(kp == 1),
                                    perf_mode=DR))
                        ex2 = ex2p.tile([128, 2 * B], bf16, tag="ex2")
                        chain("act", nc.scalar.activation(
                            out=ex2[:], in_=mp2[:], func=AF.Exp, scale=S256))
                        chain("dve", nc.vector.tensor_reduce(
                            out=fsacc[b][:, d - APE:d - APE + 1], in_=ex2[:],
                            axis=AX.X, op=OP.add))
                if d == 1:
                    emit_gather_dma()
                    emit_xnorm()
                if d == 3:
                    emit_gather_dots()

            flush_ones(len(exq))
            xinv = emit_xinv()

            # ---- PE-side sum row out ----
            sumrow = sm.tile([1, B], f32, name="sumrow")
            chain("act", nc.scalar.activation(out=sumrow[:], in_=sumP[:],
                                              func=AF.Copy))
            chain("syn", nc.sync.dma_start(out=out2[:], in_=sumrow[:]))

            # ---- target logit + DVE-side sums out ----
            tg0 = sm.tile([128, 4], f32, name="tg0")
            chain("dve", nc.vector.tensor_tensor(tg0[:], dots[:], xinv[:],
                                                 OP.mult))
            outt = sm.tile([128, 8], f32, name="outt")
            chain("dve", nc.vector.tensor_tensor(outt[:, 4:8], tg0[:],
                                                 yvs[:], OP.mult))
            for b in range(4):
                chain("dve", nc.vector.tensor_reduce(
                    out=outt[:, b:b + 1], in_=fsacc[b][:], axis=AX.X,
                    op=OP.add))
            chain("syn", nc.sync.dma_start(out=out[:], in_=outt[:]))

    return nc


_CACHE = {}


def _quant8(a):
    import ml_dtypes
    return np.clip(a * QS, -240.0, 240.0).astype(ml_dtypes.float8_e4m3)


def make_in_maps(x, y, W):
    x = np.ascontiguousarray(np.asarray(x, dtype=np.float32))
    y = np.asarray(y).astype(np.int64)
    W = np.asarray(W, dtype=np.float32)

    wnrm = np.sqrt(np.einsum("cd,cd->c", W, W, dtype=np.float64))
    Wn = W / np.maximum(wnrm, 1e-12)[:, None].astype(np.float32)
    xnrm = np.sqrt(np.einsum("bd,bd->b", x, x, dtype=np.float64))
    xh = (x / np.maximum(xnrm, 1e-12)[:, None]).astype(np.float32)

    # xt: [128p, 4k, 512b] fp8 = xhat.T scaled
    xt8 = np.ascontiguousarray(
        _quant8(xh).T.reshape(4, 128, B).transpose(1, 0, 2).reshape(128,
                                                                    4 * B))
    in_maps = []
    for i in range(NCORES):
        c0 = i * CL
        Wsh = Wn[c0:c0 + CL]                                 # [CL, D] f32
        Wpad = np.zeros((CPAD, D), dtype=np.float32)
        Wpad[:CL] = Wsh
        # [128p, 8sc, 2j, 4k, 512c] column-major chunk layout
        wt8 = _quant8(
            Wpad.reshape(NSC, 2, 512, 4, 128).transpose(4, 0, 1, 3, 2)
        ).reshape(128, NSC * 4096)
        yloc = np.clip(y - c0, 0, CL - 1).astype(np.int32)
        valid = ((y >= c0) & (y < c0 + CL)).astype(np.float32)
        in_maps.append({
            "wt": np.ascontiguousarray(wt8),
            "xt": xt8,
            "x": x,
            "wn": np.ascontiguousarray(Wsh),
            "yi": np.ascontiguousarray(yloc.reshape(4, 128).T),
            "yv": np.ascontiguousarray(valid.reshape(4, 128).T),
        })
    return in_maps


def kernel(x, y, W, _trace=False):
    from concourse.bass_utils import run_bass_kernel_spmd
    if "nc" not in _CACHE:
        _CACHE["nc"] = build()
    in_maps = make_in_maps(x, y, W)
    res = run_bass_kernel_spmd(_CACHE["nc"], in_maps, list(range(NCORES)),
                               trace=_trace)
    fs = np.zeros(B, dtype=np.float64)
    tg = np.zeros(B, dtype=np.float64)
    for i in range(NCORES):
        o = np.asarray(res.results[i]["out"], dtype=np.float64)   # [128, 8]
        o2 = np.asarray(res.results[i]["out2"], dtype=np.float64)  # [1, B]
        fs += o[:, 0:4].T.reshape(B) + o2[0]
        tg += o[:, 4:8].T.reshape(B)
    fs -= float(NCORES * NPAD)          # zero-pad classes contribute exp(0)=1
    t = np.clip(tg, -1.0 + EPS, 1.0 - EPS)
    num = S * np.cos(np.arccos(t) + MARG)
    den = np.exp(num) + fs - np.exp(S * tg)
    loss = -np.mean(num - np.log(den))
    val = np.float32(loss)
    if _trace:
        return val, res
    return val


# revision 16
# speedup vs baseline: 1.1073x; 1.1073x over previous
"""ArcFace loss on 8 TRN2 NeuronCores — class-parallel (C=64000 over 8 cores).

No device collectives: each core emits tiny partials (its local exp-sums
over classes and masked target logit per batch row); the host
gathers/unshards the 8 partials and finishes the O(B) scalar loss math
(arccos/cos/log over 512 values) — the same data an AllReduce would
have exchanged.

Per core (C_local=8000 padded to 8192 = 8 superchunks * 1024):
  - host pre-normalizes W rows and x rows, transposes, scales by 16 and
    quantizes to fp8(e4m3); theta tiles [128b, 1024c] come from fp8
    DoubleRow matmuls (256-deep contraction, 4 per tile) into
    2-PSUM-bank f32 tiles
  - exp(0.25*mp) = exp(S*theta) on ACT to bf16, one instr per 2-bank tile
  - class-sum per tile on DVE: tensor_scalar(mult 1.0) with f32
    accum_out and a bf16 dummy out (all-16-bit operands -> 2x mode);
    optionally tensor_reduce fallback
  - x norms (for the exact target logit only): squares on GpSimd,
    reduce on DVE (late in the chain, off the critical path)
  - target logits: indirect-DMA gather of normalized W rows (f32),
    multiply on GpSimd + reduce on DVE, masked by ownership
  - pad classes (192, in the last superchunk) contribute exp(0)=1 each
Host: fs = sum_i fs_i - 8*192; t = sum_i tgt_i; num = S*cos(arccos(t)+M);
      loss = -mean(num - log(exp(num) + fs - exp(S*t))) in float64.
"""

import json
import math

import numpy as np

S = 64.0
MARG = 0.5
EPS = 1e-7
B, D, C = 512, 512, 64000
NCORES = 8
CL = C // NCORES            # 8000
NSC = 8                     # superchunks of 1024 classes
SCW = 1024
CPAD = NSC * SCW            # 8192
NPAD = CPAD - CL            # 192 zero-pad classes per core
QS = 16.0                   # fp8 pre-scale for both xhat and What
S256 = S / (QS * QS)
TS_ACCUM = True             # class-sums via tensor_scalar+accum_out

_MAX_WAITS = 1


def _split_waits(bir_bytes, max_waits=_MAX_WAITS):
    """walrus in this env rejects >1 sync-wait per instruction; spill extras
    onto preceding wait-only EventSemaphore instructions (same engine)."""
    m = json.loads(bir_bytes)
    uid = [0]
    for f in m.get("functions", []):
        for blk in f.get("blocks", []):
            insts = blk.get("instructions", [])
            out = []
            for i in insts:
                si = i.get("sync_info") or {}
                ws = si.get("on_wait") or []
                if len(ws) > max_waits:
                    keep = ws[-max_waits:]
                    extra = ws[:-max_waits]
                    for cs in range(0, len(extra), max_waits):
                        uid[0] += 1
                        out.append({
                            "name": f"WSPLIT-{uid[0]}",
                            "opcode": "EventSemaphore",
                            "engine": i["engine"],
                            "ins": [],
                            "outs": [],
                            "sync_info": {"on_update": [],
                                          "on_wait": extra[cs:cs + max_waits]},
                        })
                    si["on_wait"] = keep
                out.append(i)
            blk["instructions"] = out
    return json.dumps(m).encode()


def _install_birfix():
    from concourse import bass
    if getattr(bass.Bass, "_birfix_installed", False):
        return
    orig = bass.Bass.to_json_bytes

    def to_json_bytes(self, *a, **k):
        return _split_waits(orig(self, *a, **k))

    bass.Bass.to_json_bytes = to_json_bytes
    bass.Bass._birfix_installed = True


def build():
    _install_birfix()
    from concourse import bass, tile, mybir
    from concourse.tile import add_dep_helper

    f32 = mybir.dt.float32
    bf16 = mybir.dt.bfloat16
    fp8 = mybir.dt.float8e4
    i32 = mybir.dt.int32
    AX = mybir.AxisListType
    OP = mybir.AluOpType
    AF = mybir.ActivationFunctionType
    DR = mybir.MatmulPerfMode.DoubleRow

    nc = bass.Bass("TRN2", target_bir_lowering=False, debug=False,
                   num_devices=NCORES)
    wt = nc.declare_dram_parameter("wt", [128, NSC * 4096], fp8,
                                   isOutput=False)
    xt = nc.declare_dram_parameter("xt", [128, 4 * B], fp8, isOutput=False)
    xx = nc.declare_dram_parameter("x", [B, D], f32, isOutput=False)
    wn = nc.declare_dram_parameter("wn", [CL, D], f32, isOutput=False)
    yi = nc.declare_dram_parameter("yi", [128, 4], i32, isOutput=False)
    yv = nc.declare_dram_parameter("yv", [128, 4], f32, isOutput=False)
    out = nc.declare_dram_parameter("out", [128, 8], f32, isOutput=True)

    last = {}

    def chain(key, inst):
        if key in last:
            add_dep_helper(inst.ins, last[key].ins, False, f"{key} order")
        last[key] = inst
        return inst

    with tile.TileContext(nc) as tc:
        with tc.tile_pool(name="big", bufs=1) as big, \
             tc.tile_pool(name="sm", bufs=1) as sm, \
             tc.tile_pool(name="ex2p", bufs=4) as ex2p, \
             tc.tile_pool(name="mp2p", bufs=4, space="PSUM") as mp2p:

            # ---- input tiles: xt + W stream first, gather inputs early ----
            xtile = big.tile([128, 4 * B], fp8, name="xtile")
            xr = [sm.tile([128, D], f32, name=f"xr{t}") for t in range(4)]
            idx = sm.tile([128, 4], i32, name="idx")
            yvs = sm.tile([128, 4], f32, name="yvs")
            wtile = [big.tile([128, 4096], fp8, name=f"wt{d}")
                     for d in range(NSC)]

            chain("syn", nc.sync.dma_start(out=xtile[:], in_=xt[:]))
            chain("syn", nc.sync.dma_start(out=idx[:], in_=yi[:]))
            chain("syn", nc.sync.dma_start(out=yvs[:], in_=yv[:]))
            for d in range(NSC):
                chain("syn", nc.sync.dma_start(
                    out=wtile[d][:], in_=wt[:, 4096 * d:4096 * (d + 1)]))
            for t in range(4):
                chain("syn", nc.sync.dma_start(
                    out=xr[t][:], in_=xx[128 * t:128 * (t + 1), :]))

            # views: xv [128k, kt, b];  wv [128k, j, kt, c]
            xv = xtile.rearrange("p (k b) -> p k b", k=4)
            wv = [wtile[d].rearrange("p (s k c) -> p s k c", s=2, k=4)
                  for d in range(NSC)]

            # ---- gather + x-norm inputs (GpSimd work, DVE sums go late) --
            wsel = sm.tile([128, 4, D], f32, name="wsel")
            xnsq = sm.tile([128, 4], f32, name="xnsq")
            dots = sm.tile([128, 4], f32, name="dots")
            dum = [sm.tile([128, D], f32, name=f"dum{i}") for i in range(2)]

            def emit_gather_dma():
                for t in range(4):
                    chain("gps", nc.gpsimd.indirect_dma_start(
                        out=wsel[:, t, :], out_offset=None, in_=wn[:],
                        in_offset=bass.IndirectOffsetOnAxis(
                            ap=idx[:, t:t + 1], axis=0)))

            gprod = [sm.tile([128, 4, D], f32, name="xprod"),
                     sm.tile([128, 4, D], f32, name="wprod")]

            def emit_gps_mults():
                for t in range(4):
                    chain("gps", nc.gpsimd.tensor_tensor(
                        gprod[0][:, t, :], xr[t][:], xr[t][:], OP.mult))
                    chain("gps", nc.gpsimd.tensor_tensor(
                        gprod[1][:, t, :], xr[t][:], wsel[:, t, :], OP.mult))

            def emit_small_reduces():
                for t in range(4):
                    chain("dve", nc.vector.tensor_reduce(
                        out=xnsq[:, t:t + 1], in_=gprod[0][:, t, :],
                        axis=AX.X, op=OP.add))
                    chain("dve", nc.vector.tensor_reduce(
                        out=dots[:, t:t + 1], in_=gprod[1][:, t, :],
                        axis=AX.X, op=OP.add))

            # ---- main loop: [b, c] 2-bank tiles ----
            fsacc = [sm.tile([128, NSC], f32, name=f"fsacc{b}")
                     for b in range(4)]
            sdum = sm.tile([128, 2 * B], bf16, name="sdum")
            for d in range(NSC):
                for b in range(4):
                    mp2 = mp2p.tile([128, 2 * B], f32, tag="mp")
                    for kp in range(2):
                        for j in range(2):
                            chain("pe", nc.tensor.matmul(
                                mp2[:, 512 * j:512 * (j + 1)],
                                lhsT=xv[:, 2 * kp:2 * kp + 2,
                                        128 * b:128 * (b + 1)],
                                rhs=wv[d][:, j, 2 * kp:2 * kp + 2, :],
                                start=(kp == 0), stop=(kp == 1),
                                perf_mode=DR))
                    ex2 = ex2p.tile([128, 2 * B], bf16, tag="ex2")
                    chain("act", nc.scalar.activation(
                        out=ex2[:], in_=mp2[:], func=AF.Exp, scale=S256))
                    if TS_ACCUM:
                        chain("dve", nc.vector.tensor_scalar(
                            sdum[:], ex2[:], 1.0, 0.0, OP.mult, OP.add,
                            accum_out=fsacc[b][:, d:d + 1]))
                    else:
                        chain("dve", nc.vector.tensor_reduce(
                            out=fsacc[b][:, d:d + 1], in_=ex2[:],
                            axis=AX.X, op=OP.add))
                if d == 1:
                    emit_gather_dma()
                if d == 3:
                    emit_gps_mults()
                if d == 6:
                    emit_small_reduces()

            # ---- x inverse norm (ACT), target logit, pack + out ----
            xnm = sm.tile([128, 4], f32, name="xnm")
            chain("dve", nc.vector.tensor_scalar_max(xnm[:], xnsq[:], 1e-30))
            lnx = sm.tile([128, 4], f32, name="lnx")
            chain("act", nc.scalar.activation(out=lnx[:], in_=xnm[:],
                                              func=AF.Ln))
            xinv = sm.tile([128, 4], f32, name="xinv")
            chain("act", nc.scalar.activation(out=xinv[:], in_=lnx[:],
                                              func=AF.Exp, scale=-0.5))

            tg0 = sm.tile([128, 4], f32, name="tg0")
            chain("dve", nc.vector.tensor_tensor(tg0[:], dots[:], xinv[:],
                                                 OP.mult))
            outt = sm.tile([128, 8], f32, name="outt")
            chain("dve", nc.vector.tensor_tensor(outt[:, 4:8], tg0[:],
                                                 yvs[:], OP.mult))
            for b in range(4):
                chain("dve", nc.vector.tensor_reduce(
                    out=outt[:, b:b + 1], in_=fsacc[b][:], axis=AX.X,
                    op=OP.add))
            chain("syn", nc.sync.dma_start(out=out[:], in_=outt[:]))

    return nc


_CACHE = {}


def _quant8(a):
    import ml_dtypes
    return np.clip(a * QS, -240.0, 240.0).astype(ml_dtypes.float8_e4m3)


def make_in_maps(x, y, W):
    x = np.ascontiguousarray(np.asarray(x, dtype=np.float32))
    y = np.asarray(y).astype(np.int64)
    W = np.asarray(W, dtype=np.float32)

    wnrm = np.sqrt(np.einsum("cd,cd->c", W, W, dtype=np.float64))
    Wn = W / np.maximum(wnrm, 1e-12)[:, None].astype(np.float32)
    xnrm = np.sqrt(np.einsum("bd,bd->b", x, x, dtype=np.float64))
    xh = (x / np.maximum(xnrm, 1e-12)[:, None]).astype(np.float32)

    # xt: [128p, 4k, 512b] fp8 = xhat.T scaled
    xt8 = np.ascontiguousarray(
        _quant8(xh).T.reshape(4, 128, B).transpose(1, 0, 2).reshape(128,
                                                                    4 * B))
    in_maps = []
    for i in range(NCORES):
        c0 = i * CL
        Wsh = Wn[c0:c0 + CL]                                 # [CL, D] f32
        Wpad = np.zeros((CPAD, D), dtype=np.float32)
        Wpad[:CL] = Wsh
        # [128p, 8sc, 2j, 4k, 512c] column-major chunk layout
        wt8 = _quant8(
            Wpad.reshape(NSC, 2, 512, 4, 128).transpose(4, 0, 1, 3, 2)
        ).reshape(128, NSC * 4096)
        yloc = np.clip(y - c0, 0, CL - 1).astype(np.int32)
        valid = ((y >= c0) & (y < c0 + CL)).astype(np.float32)
        in_maps.append({
            "wt": np.ascontiguousarray(wt8),
            "xt": xt8,
            "x": x,
            "wn": np.ascontiguousarray(Wsh),
            "yi": np.ascontiguousarray(yloc.reshape(4, 128).T),
            "yv": np.ascontiguousarray(valid.reshape(4, 128).T),
        })
    return in_maps


def kernel(x, y, W, _trace=False):
    from concourse.bass_utils import run_bass_kernel_spmd
    if "nc" not in _CACHE:
        _CACHE["nc"] = build()
    in_maps = make_in_maps(x, y, W)
    res = run_bass_kernel_spmd(_CACHE["nc"], in_maps, list(range(NCORES)),
                               trace=_trace)
    fs = np.zeros(B, dtype=np.float64)
    tg = np.zeros(B, dtype=np.float64)
    for i in range(NCORES):
        o = np.asarray(res.results[i]["out"], dtype=np.float64)   # [128, 8]
        fs += o[:, 0:4].T.reshape(B)
        tg += o[:, 4:8].T.reshape(B)
    fs -= float(NCORES * NPAD)          # zero-pad classes contribute exp(0)=1
    t = np.clip(tg, -1.0 + EPS, 1.0 - EPS)
    num = S * np.cos(np.arccos(t) + MARG)
    den = np.exp(num) + fs - np.exp(S * tg)
    loss = -np.mean(num - np.log(den))
    val = np.float32(loss)
    if _trace:
        return val, res
    return val
